# revision 1
# baseline (speedup 1.0000x reference)
"""Bass kernel for nn_ArithmeticGreyboxModule (scatter_memory, 8 cores).

The reference blends the input carrier with a "symbolic" copy that differs
from the input only inside sequence rows 0..19 (the protected register
rows) at complex freq bin 0 — i.e. flat columns 0..1 — plus, for the
START token, all of rows 0..19. Everywhere else blended == input up to
one ulp of ((1-b)*x + b*x) vs x.

Strategy: shard batch dim B=8 across the 8 NeuronCores (one batch each).
The token-dependent region (rows 0..19, all 258 cols, per batch) is
computed exactly on the host (tiny: 20x258 floats per core) and shipped
as a second input. Each core's device program is pure DMA:
  out[20:, :]  = x[20:, :]     (33.77 MB DRAM->DRAM copy)
  out[:20, :]  = strip         (20 KB DRAM->DRAM copy)
which is the memory roofline for this problem (read+write of the shard).
"""

import sys

import numpy as np

for _p in ("/opt/trn_rl_repo",):
    if _p not in sys.path:
        sys.path.insert(0, _p)

import concourse.bass as bass
import concourse.mybir as mybir
from concourse.bass_utils import run_bass_kernel_spmd

B, T, C = 8, 32768, 258
N_CORES = 8
STRIP = 20  # NUM_PROTECTED rows; every token-dependent write lands in rows < 20

DIGIT_TOKENS = set(range(1, 11))
PLUS, MINUS, EQUALS, START = 11, 12, 13, 0

_NC_CACHE = {}


def build_nc():
    """Per-core Bass program: two disjoint DRAM->DRAM copies."""
    nc = bass.Bass()
    x = nc.declare_dram_parameter("x", [T, C], mybir.dt.float32, isOutput=False)
    strip = nc.declare_dram_parameter(
        "strip", [STRIP, C], mybir.dt.float32, isOutput=False
    )
    out = nc.declare_dram_parameter("out", [T, C], mybir.dt.float32, isOutput=True)

    with nc.Block() as block, nc.semaphore("dma_sem") as dma_sem:

        @block.sync
        def _(sync: bass.BassEngine):
            sync.dma_start(out=out[STRIP:, :], in_=x[STRIP:, :]).then_inc(dma_sem, 16)
            sync.dma_start(out=out[:STRIP, :], in_=strip[:, :]).then_inc(dma_sem, 16)
            sync.wait_ge(dma_sem, 32)

    return nc


def _get_nc():
    if "nc" not in _NC_CACHE:
        _NC_CACHE["nc"] = build_nc()
    return _NC_CACHE["nc"]


def _host_strip(x_strip: np.ndarray, src_token: int, blend: np.float32) -> np.ndarray:
    """Exact blended output for rows 0..19, mirroring reference._inject.

    x_strip: (B, STRIP, C) float32. Flat layout: cols (2f, 2f+1) are the
    real/imag parts of freq bin f; 'complex index [reg, 0]' == cols 0..1
    of row reg.
    """
    sym = x_strip.copy()
    st = int(src_token)
    if st == START:
        sym[:, :STRIP, :] = 0.0
    if st in DIGIT_TOKENS:
        dv = (st - 1) % 10
        sym[:, 2:12, 0:2] = 0.0
        sym[:, 2 + dv, 0] = 1.0
        sym[:, 2 + dv, 1] = 0.0
    if st == PLUS:
        sym[:, 1, 0] = 1.0
        sym[:, 1, 1] = 0.0
    if st == MINUS:
        sym[:, 1, 0] = -1.0
        sym[:, 1, 1] = 0.0
    if st == EQUALS:
        sym[:, 14, 0:2] = 0.0
        sym[:, 15, 0:2] = 0.0
        sym[:, 16, 0:2] = 0.0
        sym[:, 1, 0:2] = 0.0
        sym[:, 2:12, 0:2] = 0.0
    one = np.float32(1.0)
    return ((one - blend) * x_strip + blend * sym).astype(np.float32)


def make_in_maps(inputs: dict) -> list[dict]:
    x = np.ascontiguousarray(
        np.asarray(inputs["carrier_freq_flat"], dtype=np.float32)
    ).reshape(B, T, C)
    src = inputs.get("src_token")
    tgt = inputs.get("tgt_token")
    if src is None or tgt is None:
        strip = np.ascontiguousarray(x[:, :STRIP, :])
    else:
        sb = np.float32(np.asarray(inputs["symbolic_blend"], dtype=np.float32))
        blend = np.float32(1.0) / (np.float32(1.0) + np.exp(-sb, dtype=np.float32))
        strip = _host_strip(np.ascontiguousarray(x[:, :STRIP, :]), int(src), blend)
    return [{"x": x[b], "strip": strip[b]} for b in range(B)]


def kernel(**inputs) -> np.ndarray:
    in_maps = make_in_maps(inputs)
    res = run_bass_kernel_spmd(_get_nc(), in_maps, list(range(N_CORES)))
    return np.stack([res.results[b]["out"] for b in range(B)], axis=0)


# revision 3
# speedup vs baseline: 1.7851x; 1.7851x over previous
"""Bass kernel for nn_ArithmeticGreyboxModule (scatter_memory, 8 cores).

The reference blends the input carrier with a "symbolic" copy that differs
from the input only inside sequence rows 0..19 (the protected register
rows) at complex freq bin 0 — i.e. flat columns 0..1 — plus, for the
START token, all of rows 0..19. Everywhere else blended == input up to
one ulp of ((1-b)*x + b*x) vs x.

Strategy: shard batch dim B=8 across the 8 NeuronCores (one batch each).
The token-dependent region (rows 0..19, all 258 cols, per batch) is
computed exactly on the host (tiny: 20x258 floats per core) and shipped
as a second input. Each core's device program is pure DMA:
  out[20:, :]  = x[20:, :]     (33.77 MB DRAM->DRAM copy)
  out[:20, :]  = strip         (20 KB DRAM->DRAM copy)
which is the memory roofline for this problem (read+write of the shard).
"""

import sys

import numpy as np

for _p in ("/opt/trn_rl_repo",):
    if _p not in sys.path:
        sys.path.insert(0, _p)

import concourse.bass as bass
import concourse.mybir as mybir
from concourse.bass_utils import run_bass_kernel_spmd

B, T, C = 8, 32768, 258
N_CORES = 8
STRIP = 20  # NUM_PROTECTED rows; every token-dependent write lands in rows < 20

DIGIT_TOKENS = set(range(1, 11))
PLUS, MINUS, EQUALS, START = 11, 12, 13, 0

_NC_CACHE = {}


def build_nc():
    """Per-core Bass program: pure DRAM->DRAM copies.

    The copy region [STRIP, T) is split in thirds across the three
    DMA-capable initiators (sync + scalar HWDGE rings, gpsimd SWDGE), so
    three descriptor queues feed the SDMA engines concurrently — one
    queue alone leaves the engines at half line rate.  Block drains for
    gpsimd are skipped (completion is guaranteed by the dma semaphore).
    """
    nc = bass.Bass()
    x = nc.declare_dram_parameter("x", [T, C], mybir.dt.float32, isOutput=False)
    strip = nc.declare_dram_parameter(
        "strip", [STRIP, C], mybir.dt.float32, isOutput=False
    )
    out = nc.declare_dram_parameter("out", [T, C], mybir.dt.float32, isOutput=True)

    b0 = STRIP
    b1 = STRIP + (T - STRIP) // 3
    b2 = STRIP + 2 * (T - STRIP) // 3
    MDLD = 2**17  # 128KB descriptors

    with (
        nc.Block(no_gpsimd_drain=True) as block,
        nc.semaphore("sp_sem") as sp_sem,
        nc.semaphore("act_sem") as act_sem,
        nc.semaphore("gp_sem") as gp_sem,
    ):

        @block.sync
        def _(sync: bass.BassEngine):
            sync.dma_start(
                out=out[b0:b1, :], in_=x[b0:b1, :], max_dma_last_dim=MDLD
            ).then_inc(sp_sem, 16)
            sync.dma_start(out=out[:STRIP, :], in_=strip[:, :]).then_inc(sp_sem, 16)
            sync.wait_ge(sp_sem, 32)

        @block.scalar
        def _(scalar: bass.BassEngine):
            scalar.dma_start(
                out=out[b1:b2, :], in_=x[b1:b2, :], max_dma_last_dim=MDLD
            ).then_inc(act_sem, 16)
            scalar.wait_ge(act_sem, 16)

        @block.gpsimd
        def _(gpsimd: bass.BassEngine):
            gpsimd.dma_start(
                out=out[b2:, :], in_=x[b2:, :], max_dma_last_dim=MDLD
            ).then_inc(gp_sem, 16)
            gpsimd.wait_ge(gp_sem, 16)

    return nc


def _get_nc():
    if "nc" not in _NC_CACHE:
        _NC_CACHE["nc"] = build_nc()
    return _NC_CACHE["nc"]


def _host_strip(x_strip: np.ndarray, src_token: int, blend: np.float32) -> np.ndarray:
    """Exact blended output for rows 0..19, mirroring reference._inject.

    x_strip: (B, STRIP, C) float32. Flat layout: cols (2f, 2f+1) are the
    real/imag parts of freq bin f; 'complex index [reg, 0]' == cols 0..1
    of row reg.
    """
    sym = x_strip.copy()
    st = int(src_token)
    if st == START:
        sym[:, :STRIP, :] = 0.0
    if st in DIGIT_TOKENS:
        dv = (st - 1) % 10
        sym[:, 2:12, 0:2] = 0.0
        sym[:, 2 + dv, 0] = 1.0
        sym[:, 2 + dv, 1] = 0.0
    if st == PLUS:
        sym[:, 1, 0] = 1.0
        sym[:, 1, 1] = 0.0
    if st == MINUS:
        sym[:, 1, 0] = -1.0
        sym[:, 1, 1] = 0.0
    if st == EQUALS:
        sym[:, 14, 0:2] = 0.0
        sym[:, 15, 0:2] = 0.0
        sym[:, 16, 0:2] = 0.0
        sym[:, 1, 0:2] = 0.0
        sym[:, 2:12, 0:2] = 0.0
    one = np.float32(1.0)
    return ((one - blend) * x_strip + blend * sym).astype(np.float32)


def make_in_maps(inputs: dict) -> list[dict]:
    x = np.ascontiguousarray(
        np.asarray(inputs["carrier_freq_flat"], dtype=np.float32)
    ).reshape(B, T, C)
    src = inputs.get("src_token")
    tgt = inputs.get("tgt_token")
    if src is None or tgt is None:
        strip = np.ascontiguousarray(x[:, :STRIP, :])
    else:
        sb = np.float32(np.asarray(inputs["symbolic_blend"], dtype=np.float32))
        blend = np.float32(1.0) / (np.float32(1.0) + np.exp(-sb, dtype=np.float32))
        strip = _host_strip(np.ascontiguousarray(x[:, :STRIP, :]), int(src), blend)
    return [{"x": x[b], "strip": strip[b]} for b in range(B)]


def kernel(**inputs) -> np.ndarray:
    in_maps = make_in_maps(inputs)
    res = run_bass_kernel_spmd(_get_nc(), in_maps, list(range(N_CORES)))
    return np.stack([res.results[b]["out"] for b in range(B)], axis=0)


# revision 6
# speedup vs baseline: 1.8230x; 1.0213x over previous
"""Bass kernel for nn_ArithmeticGreyboxModule (scatter_memory, 8 cores).

The reference blends the input carrier with a "symbolic" copy that differs
from the input only inside sequence rows 0..19 (the protected register
rows) at complex freq bin 0 — i.e. flat columns 0..1 — plus, for the
START token, all of rows 0..19. Everywhere else blended == input up to
one ulp of ((1-b)*x + b*x) vs x.

Strategy: shard batch dim B=8 across the 8 NeuronCores (one batch each).
The token-dependent region (rows 0..19, all 258 cols, per batch) is
computed exactly on the host (tiny: 20x258 floats per core) and shipped
as a second input. Each core's device program is pure DMA:
  out[20:, :]  = x[20:, :]     (33.77 MB DRAM->DRAM copy)
  out[:20, :]  = strip         (20 KB DRAM->DRAM copy)
which is the memory roofline for this problem (read+write of the shard).
"""

import sys

import numpy as np

for _p in ("/opt/trn_rl_repo",):
    if _p not in sys.path:
        sys.path.insert(0, _p)

import concourse.bass as bass
import concourse.mybir as mybir
from concourse.bass_utils import run_bass_kernel_spmd

try:  # bass_utils needs this module when tracing (BASS_TRACE=1); the
    import antenv.axon_hooks  # noqa: F401  # image may not ship it.
except ImportError:
    import types

    import antenv

    _hooks = types.ModuleType("antenv.axon_hooks")
    _hooks._hook = None

    def _set_hook(h):
        _hooks._hook = h

    def _get_hook():
        if _hooks._hook is None:
            try:
                if "/root/.axon_site" not in sys.path:
                    sys.path.insert(0, "/root/.axon_site")
                from trn_agent_boot.trn_boot import _ntff_profile_via_ctypes

                _hooks._hook = _ntff_profile_via_ctypes(
                    "/opt/axon/libaxon_pjrt.so"
                )
            except Exception:
                return None
        return _hooks._hook

    _hooks.set_axon_ntff_profile_hook = _set_hook
    _hooks.get_axon_ntff_profile_hook = _get_hook
    sys.modules["antenv.axon_hooks"] = _hooks
    antenv.axon_hooks = _hooks

B, T, C = 8, 32768, 258
N_CORES = 8
STRIP = 20  # NUM_PROTECTED rows; every token-dependent write lands in rows < 20

DIGIT_TOKENS = set(range(1, 11))
PLUS, MINUS, EQUALS, START = 11, 12, 13, 0

_NC_CACHE = {}


def build_nc():
    """Per-core Bass program: pure DRAM->DRAM copies.

    The copy region [STRIP, T) is split in thirds across the three
    DMA-capable initiators (sync + scalar HWDGE rings, gpsimd SWDGE), so
    three descriptor queues feed the SDMA engines concurrently — one
    queue alone leaves the engines at half line rate.  Block drains for
    gpsimd are skipped (completion is guaranteed by the dma semaphore).
    """
    nc = bass.Bass()
    x = nc.declare_dram_parameter("x", [T, C], mybir.dt.float32, isOutput=False)
    strip = nc.declare_dram_parameter(
        "strip", [STRIP, C], mybir.dt.float32, isOutput=False
    )
    out = nc.declare_dram_parameter("out", [T, C], mybir.dt.float32, isOutput=True)

    # Equal thirds, with the scalar ring's share biased +400 rows: its
    # queue consistently drains a few percent faster than the other two.
    third = (T - STRIP) // 3
    b0 = STRIP
    b1 = STRIP + third - 200
    b2 = b1 + third + 400

    with (
        nc.Block(no_gpsimd_drain=True) as block,
        nc.semaphore("sp_sem") as sp_sem,
        nc.semaphore("act_sem") as act_sem,
        nc.semaphore("gp_sem") as gp_sem,
    ):

        @block.sync
        def _(sync: bass.BassEngine):
            sync.dma_start(out=out[b0:b1, :], in_=x[b0:b1, :]).then_inc(sp_sem, 16)
            sync.dma_start(out=out[:STRIP, :], in_=strip[:, :]).then_inc(sp_sem, 16)
            sync.wait_ge(sp_sem, 32)

        @block.scalar
        def _(scalar: bass.BassEngine):
            scalar.dma_start(out=out[b1:b2, :], in_=x[b1:b2, :]).then_inc(act_sem, 16)
            scalar.wait_ge(act_sem, 16)

        @block.gpsimd
        def _(gpsimd: bass.BassEngine):
            gpsimd.dma_start(out=out[b2:, :], in_=x[b2:, :]).then_inc(gp_sem, 16)
            gpsimd.wait_ge(gp_sem, 16)

    return nc


def _get_nc():
    if "nc" not in _NC_CACHE:
        _NC_CACHE["nc"] = build_nc()
    return _NC_CACHE["nc"]


def _host_strip(x_strip: np.ndarray, src_token: int, blend: np.float32) -> np.ndarray:
    """Exact blended output for rows 0..19, mirroring reference._inject.

    x_strip: (B, STRIP, C) float32. Flat layout: cols (2f, 2f+1) are the
    real/imag parts of freq bin f; 'complex index [reg, 0]' == cols 0..1
    of row reg.
    """
    sym = x_strip.copy()
    st = int(src_token)
    if st == START:
        sym[:, :STRIP, :] = 0.0
    if st in DIGIT_TOKENS:
        dv = (st - 1) % 10
        sym[:, 2:12, 0:2] = 0.0
        sym[:, 2 + dv, 0] = 1.0
        sym[:, 2 + dv, 1] = 0.0
    if st == PLUS:
        sym[:, 1, 0] = 1.0
        sym[:, 1, 1] = 0.0
    if st == MINUS:
        sym[:, 1, 0] = -1.0
        sym[:, 1, 1] = 0.0
    if st == EQUALS:
        sym[:, 14, 0:2] = 0.0
        sym[:, 15, 0:2] = 0.0
        sym[:, 16, 0:2] = 0.0
        sym[:, 1, 0:2] = 0.0
        sym[:, 2:12, 0:2] = 0.0
    one = np.float32(1.0)
    return ((one - blend) * x_strip + blend * sym).astype(np.float32)


def make_in_maps(inputs: dict) -> list[dict]:
    x = np.ascontiguousarray(
        np.asarray(inputs["carrier_freq_flat"], dtype=np.float32)
    ).reshape(B, T, C)
    src = inputs.get("src_token")
    tgt = inputs.get("tgt_token")
    if src is None or tgt is None:
        strip = np.ascontiguousarray(x[:, :STRIP, :])
    else:
        sb = np.float32(np.asarray(inputs["symbolic_blend"], dtype=np.float32))
        blend = np.float32(1.0) / (np.float32(1.0) + np.exp(-sb, dtype=np.float32))
        strip = _host_strip(np.ascontiguousarray(x[:, :STRIP, :]), int(src), blend)
    return [{"x": x[b], "strip": strip[b]} for b in range(B)]


def kernel(**inputs) -> np.ndarray:
    in_maps = make_in_maps(inputs)
    res = run_bass_kernel_spmd(_get_nc(), in_maps, list(range(N_CORES)))
    return np.stack([res.results[b]["out"] for b in range(B)], axis=0)
